# revision 1
# baseline (speedup 1.0000x reference)
"""Trainium2 Bass kernel: AttentionBlock (GroupNorm + cross-attention + residual).

Sharding: data-parallel over batch. b=8 maps 1:1 onto the 8 NeuronCores;
each core computes its whole batch item, no collectives.

Per-core algorithm (x:[512,4096], ctx:[768,256]):
  - GroupNorm(x) is folded into the Q projection: gn(x) = x*A_c + B_c with
    per-channel A,B derived from bn_stats group statistics, so
    q = (wqT*A).T @ x + (wq@B + bq); no elementwise pass over x.
  - GroupNorm(ctx) is materialized directly (small), then k = wkv_k @ gnc,
    vT = gnc.T @ wkv_v computed directly transposed. Each head's vT block
    is 128 wide with a ones column and zero padding (even head
    [v(64)|1|0*63], odd head [1|0*63|v(64)]) so the attention-value matmul
    yields the softmax denominator for free and odd-head data lands on
    psum partitions 64..127 (PE outputs must start 32-aligned).
  - dots are computed per head in [s, t] layout (lhsT=k, rhs=q), exp on the
    scalar engine with the 1/dh scaling folded in (|dots|<1 so no
    max-subtraction is needed), AV accumulates straight in PSUM. The
    denominator rows are broadcast over partitions with two K=1 matmuls,
    reciprocated on all 128 lanes with reciprocal_approx_fast, then one
    tensor_tensor multiply per head normalizes.
  - All big matmuls run as float32r (1 cycle/row at N>=256).
"""

import sys

import numpy as np

sys.path.insert(0, "/opt/trn_rl_repo")

import concourse.bacc as bacc
import concourse.bass as bass
import concourse.mybir as mybir
import concourse.tile as tile
from concourse.bass_utils import run_bass_kernel_spmd

F32 = mybir.dt.float32
F32R = mybir.dt.float32r
AF = mybir.ActivationFunctionType
OP = mybir.AluOpType

B = 8
C = 512
L = 4096          # 64*64
CC = 768
S = 256
INNER = 512
NH = 8
DH = 64
G = 32
EPS = 1e-5
TT = 512          # t-tile
NT = L // TT      # 8
XC = C // 128     # 4
CCHUNK = CC // 128  # 6
MC = INNER // 128   # 4
NCORES = 8
SCALE2 = 1.0 / DH   # (q*dh^-.5)@(k*dh^-.5) == (q@k)/dh

# packed per-partition vector columns: [bq, bkv_k, bo, gnx_g, gnx_b, gnc_g, gnc_b]
VOFF = {"bq": 0, "bkvk": 4, "bo": 8, "gxg": 12, "gxb": 16, "gcg": 20, "gcb": 26,
        "eps": 32}
VCOLS = 36


def _r(ap):
    return ap.bitcast(F32R)


def _emit(nc, tc, d):
    sync = nc.sync
    act = nc.scalar
    dve = nc.vector
    pe = nc.tensor
    ds = bass.ds

    with tc.tile_pool(name="keep", bufs=1) as keep:
        # ---------------- persistent tiles ----------------
        xall = keep.tile([128, XC, L], F32, name="xall", tag="xall")
        wqe = keep.tile([128, XC, INNER], F32, name="wqe", tag="wqe")
        wo_sb = keep.tile([128, MC, C], F32, name="wo_sb", tag="wo_sb")
        k_sb = keep.tile([128, MC, S], F32, name="k_sb", tag="k_sb")
        # per head a 128-wide block: even [v(64)|1|0*63], odd [1|0*63|v(64)]
        vaug = [keep.tile([128, NH * 128], F32, name=f"vaug{sc}", tag=f"vaug{sc}")
                for sc in range(2)]
        vecs = keep.tile([128, VCOLS], F32, name="vecs", tag="vecs")
        bqe = keep.tile([128, MC], F32, name="bqe", tag="bqe")
        ones128 = keep.tile([128, 128], F32, name="ones128", tag="ones128")
        patd = keep.tile([128, 128], F32, name="patd", tag="patd")
        dnst = keep.tile([128, 2, TT], F32, name="dnst", tag="dnst")

        def vcol(nm, j=0):
            return vecs[:, VOFF[nm] + j:VOFF[nm] + j + 1]

        with tc.tile_pool(name="sb0", bufs=1) as sb0, \
             tc.tile_pool(name="ps0", bufs=1, space="PSUM") as ps0:

            ctx_sb = sb0.tile([128, CCHUNK, S], F32, name="ctx_sb", tag="ctx_sb")
            gnc = sb0.tile([128, CCHUNK, S], F32, name="gnc", tag="gnc")
            indall = sb0.tile([128, CCHUNK + XC, G], F32, name="indall",
                              tag="indall")
            indTall = sb0.tile([G, CC + C + INNER], F32, name="indTall",
                               tag="indTall")
            wqT_sb = sb0.tile([128, XC, INNER], F32, name="wqT_sb", tag="wqT_sb")

            # ---- DMA schedule: ctx/indicators/vecs, then x, wqT, wkv, wo ----
            sync.dma_start(ctx_sb[:, :, :],
                           d["ctx"].rearrange("(j p) s -> p j s", p=128))
            sync.dma_start(_r(indall[:, :, :]),
                           _r(d["indall"].rearrange("(j p) g -> p j g", p=128)))
            sync.dma_start(_r(indTall[:, :]), _r(d["indTall"][:, :]))
            sync.dma_start(vecs[:, :], d["vecs"][:, :])
            xv = d["x"].rearrange("(m p) l -> p m l", p=128)
            for m in range(XC):
                for half in range(2):
                    sync.dma_start(
                        _r(xall[:, m, half * 2048:(half + 1) * 2048]),
                        _r(xv[:, m, half * 2048:(half + 1) * 2048]))
            sync.dma_start(_r(wqT_sb[:, :, :]),
                           _r(d["wqT"].rearrange("(m p) o -> p m o", p=128)))
            sync.dma_start(_r(ones128[:, :]), _r(d["ones128"][:, :]))

            ind_c = [indall[:, j, :] for j in range(CCHUNK)]
            ind_x = [indall[:, CCHUNK + m, :] for m in range(XC)]
            indT_c = indTall[:, 0:CC]
            indT_x = indTall[:, CC:CC + C]
            bkvv_row = indTall[0:1, CC + C:CC + C + INNER]

            def chan_stats(src, nch, nblk, blk, ind_tiles, inv_n, tagp):
                """bn_stats per chunk -> per-channel (mean, E[x^2]) columns ->
                indicator matmul -> group (mu, E[x^2]) in SBUF [G,2]."""
                bns = []
                for j in range(nch):
                    bn = sb0.tile([128, nblk * 6], F32, name=f"bn{tagp}{j}",
                                  tag=f"bn{tagp}{j}")
                    bns.append(bn.rearrange("p (a q) -> p a q", q=6))
                # chunk-major: chases the per-chunk DMAs of x
                for j in range(nch):
                    for a in range(nblk):
                        dve.bn_stats(bns[j][:, a, :],
                                     src(j)[:, a * blk:(a + 1) * blk])
                rhs_list = []
                for j in range(nch):
                    st = sb0.tile([128, 2], F32, name=f"st{tagp}{j}",
                                  tag=f"st{tagp}{j}")
                    dve.bn_aggr(st[:, :], bns[j])
                    r2 = sb0.tile([128, 2], F32, name=f"r2{tagp}{j}",
                                  tag=f"r2{tagp}{j}")
                    dve.tensor_copy(_r(r2[:, 0:1]), st[:, 0:1])
                    dve.scalar_tensor_tensor(_r(r2[:, 1:2]), st[:, 0:1],
                                             st[:, 0:1], st[:, 1:2],
                                             op0=OP.mult, op1=OP.add)
                    rhs_list.append(r2)

                psg = ps0.tile([G, 2], F32, name=f"psg{tagp}", tag="misc", bufs=2)
                for j in range(nch):
                    pe.matmul(psg[:, :], ind_tiles[j], rhs_list[j][:, :],
                              start=(j == 0), stop=(j == nch - 1))
                gstat = sb0.tile([G, 2], F32, name=f"gstat{tagp}",
                                 tag=f"gstat{tagp}")
                act.mul(gstat[:, :], psg[:, :], inv_n)
                nvar = sb0.tile([G, 1], F32, name=f"nvar{tagp}", tag=f"nvar{tagp}")
                dve.scalar_tensor_tensor(nvar[:, :], gstat[:, 0:1],
                                         gstat[:, 0:1], gstat[:, 1:2],
                                         op0=OP.mult, op1=OP.subtract)
                sd = sb0.tile([G, 1], F32, name=f"sd{tagp}", tag=f"sd{tagp}")
                act.activation(sd[:, :], nvar[:, :], AF.Sqrt, scale=-1.0,
                               bias=vcol("eps")[0:G, :])
                rstd = sb0.tile([G, 1], F32, name=f"rstd{tagp}", tag=f"rstd{tagp}")
                dve.reciprocal(rstd[:, :], sd[:, :])
                er = sb0.tile([G, 2], F32, name=f"er{tagp}", tag=f"er{tagp}")
                dve.tensor_copy(_r(er[:, 0:1]), rstd[:, :])
                dve.tensor_copy(_r(er[:, 1:2]), gstat[:, 0:1])
                return er

            def expand_ab(er, indT, j, gam, bet, tagp):
                """[G,2] (rstd, mu) -> per-partition A=rstd*gamma, B=beta-mu*A."""
                pse = ps0.tile([128, 2], F32, name=f"pse{tagp}{j}", tag="misc",
                               bufs=2)
                pe.matmul(pse[:, :], indT[:, j * 128:(j + 1) * 128], er[:, :],
                          start=True, stop=True)
                A = sb0.tile([128, 1], F32, name=f"A{tagp}{j}", tag=f"A{tagp}{j}")
                dve.tensor_tensor(A[:, :], pse[:, 0:1], gam(j), op=OP.mult)
                Bt = sb0.tile([128, 1], F32, name=f"B{tagp}{j}", tag=f"B{tagp}{j}")
                muA = sb0.tile([128, 1], F32, name=f"muA{tagp}{j}",
                               tag=f"muA{tagp}", bufs=2)
                dve.tensor_tensor(muA[:, :], pse[:, 1:2], A[:, :], op=OP.mult)
                dve.tensor_tensor(_r(Bt[:, :]), bet(j), muA[:, :], op=OP.subtract)
                return A, Bt

            # context groupnorm (materialized)
            er_c = chan_stats(lambda j: ctx_sb[:, j, :], CCHUNK, 1, S, ind_c,
                              1.0 / (CC // G), "c")
            for j in range(CCHUNK):
                A, Bt = expand_ab(er_c, indT_c, j, lambda jj: vcol("gcg", jj),
                                  lambda jj: vcol("gcb", jj), "c")
                dve.tensor_scalar(_r(gnc[:, j, :]), ctx_sb[:, j, :], A[:, :],
                                  Bt[:, :], op0=OP.mult, op1=OP.add)

            # x stats -> fold into wq
            er_x = chan_stats(lambda m: xall[:, m, :], XC, 8, 512, ind_x,
                              1.0 / (C // G), "x")
            Bx = []
            for m in range(XC):
                A, Bt = expand_ab(er_x, indT_x, m, lambda jj: vcol("gxg", jj),
                                  lambda jj: vcol("gxb", jj), "x")
                act.activation(_r(wqe[:, m, :]), wqT_sb[:, m, :], AF.Copy,
                               scale=A[:, :])
                Bx.append(Bt)
            # bq_eff = bq + wq @ B    (tiny fp32 matmuls, N=1)
            for m in range(MC):
                psb = ps0.tile([128, 1], F32, name=f"psb{m}", tag="misc", bufs=2)
                for kc in range(XC):
                    pe.matmul(psb[:, :],
                              wqT_sb[:, kc, m * 128:(m + 1) * 128],
                              Bx[kc][:, :], start=(kc == 0), stop=(kc == XC - 1))
                dve.tensor_tensor(bqe[:, m:m + 1], psb[:, :], vcol("bq", m),
                                  op=OP.add)

            # ---- K and V^T projections ----
            psk = [ps0.tile([128, S], F32, name=f"psk{m}", tag=f"psk{m}")
                   for m in range(MC)]
            psv = [ps0.tile([128, INNER], F32, name=f"psv{sc}", tag=f"psv{sc}")
                   for sc in range(2)]
            for kc in range(CCHUNK):
                wkv_t = sb0.tile([128, 2 * INNER], F32, name=f"wkv{kc}", tag="wkv",
                                 bufs=3)
                sync.dma_start(_r(wkv_t[:, :]),
                               _r(d["wkvT"][kc * 128:(kc + 1) * 128, :]))
                for m in range(MC):
                    pe.matmul(psk[m][:, :], _r(wkv_t[:, m * 128:(m + 1) * 128]),
                              _r(gnc[:, kc, :]), start=(kc == 0),
                              stop=(kc == CCHUNK - 1))
                for sc in range(2):
                    pe.matmul(psv[sc][:, :],
                              _r(gnc[:, kc, sc * 128:(sc + 1) * 128]),
                              _r(wkv_t[:, INNER:2 * INNER]), start=(kc == 0),
                              stop=False)
            for sc in range(2):
                sync.dma_start(_r(vaug[sc][:, :]), _r(d["vaug_pat"][:, :]))
            for sc in range(2):
                pe.matmul(psv[sc][:, :], _r(ones128[0:1, :]), _r(bkvv_row),
                          start=False, stop=True)
            for m in range(MC):
                act.activation(_r(k_sb[:, m, :]), psk[m][:, :], AF.Identity,
                               bias=vcol("bkvk", m))
            for sc in range(2):
                ve = vaug[sc].rearrange("p (hp par q) -> p hp par q", par=2, q=128)
                pv = psv[sc].rearrange("p (hp par q) -> p hp par q", par=2, q=DH)
                eng = act.copy if sc == 0 else dve.tensor_copy
                eng(_r(ve[:, :, 0:1, 0:DH]), pv[:, :, 0:1, :])
                eng(_r(ve[:, :, 1:2, DH:128]), pv[:, :, 1:2, :])
            sync.dma_start(_r(patd[:, :]), _r(d["patd"][:, :]))
            sync.dma_start(_r(dnst[:, :, :]), _r(d["zeros1k"].rearrange(
                "p (a t) -> p a t", a=2)))
            sync.dma_start(_r(wo_sb[:, :, :]),
                           _r(d["woT"].rearrange("(m p) o -> p m o", p=128)))

        # ---------------- phase 1: Q / attention / out-proj ----------------
        with tc.tile_pool(name="work", bufs=1) as work, \
             tc.tile_pool(name="ps1", bufs=1, space="PSUM") as ps1:
            out_view = d["out"].rearrange("(m p) t -> p m t", p=128)
            for ti in range(NT):
                tsl = ds(ti * TT, TT)
                # Q projection for this t-tile
                q_t = work.tile([128, MC, TT], F32, name=f"q{ti}", tag="q", bufs=4)
                for m in range(MC):
                    psq = ps1.tile([128, TT], F32, name=f"psq{ti}_{m}", tag="psq",
                                   bufs=1)
                    for kc in range(XC):
                        pe.matmul(psq[:, :], _r(wqe[:, kc, m * 128:(m + 1) * 128]),
                                  _r(xall[:, kc, tsl]), start=(kc == 0),
                                  stop=(kc == XC - 1))
                    if m % 2:
                        dve.tensor_scalar_add(_r(q_t[:, m, :]), psq[:, :],
                                              bqe[:, m:m + 1])
                    else:
                        act.activation(_r(q_t[:, m, :]), psq[:, :], AF.Identity,
                                       bias=bqe[:, m:m + 1])

                avn = work.tile([128, MC, TT], F32, name=f"avn{ti}", tag="avn",
                                bufs=3)
                for p in range(NH // 2):
                    av_pair = []
                    for par in range(2):
                        h = 2 * p + par
                        m = h // 2
                        E_t = work.tile([128, 2, TT], F32, name=f"E{ti}_{h}",
                                        tag="E", bufs=4)
                        for sc in range(2):
                            psd = ps1.tile([128, TT], F32,
                                           name=f"psd{ti}_{h}_{sc}", tag="psd",
                                           bufs=2)
                            pe.matmul(psd[:, :],
                                      _r(k_sb[par * DH:(par + 1) * DH, m,
                                              sc * 128:(sc + 1) * 128]),
                                      _r(q_t[par * DH:(par + 1) * DH, m, :]),
                                      start=True, stop=True)
                            act.activation(_r(E_t[:, sc, :]), psd[:, :], AF.Exp,
                                           scale=SCALE2)
                        av = ps1.tile([128, TT], F32, name=f"av{ti}_{h}", tag="av",
                                      bufs=3)
                        for sc in range(2):
                            pe.matmul(av[:, :],
                                      _r(vaug[sc][:, h * 128:(h + 1) * 128]),
                                      _r(E_t[:, sc, :]), start=(sc == 0),
                                      stop=(sc == 1))
                        av_pair.append(av)
                    # denominator rows into the zeroed staging slab (rows 0
                    # and 64), one K=65 matmul broadcasts both halves
                    slab = p % 2
                    act.copy(_r(dnst[DH:DH + 1, slab, :]),
                             av_pair[0][DH:DH + 1, :])
                    dve.tensor_copy(_r(dnst[0:1, slab, :]), av_pair[1][0:1, :])
                    bc = ps1.tile([128, TT], F32, name=f"bc{ti}_{p}", tag="bc",
                                  bufs=1)
                    pe.matmul(bc[:, :], _r(patd[0:DH + 1, :]),
                              _r(dnst[0:DH + 1, slab, :]), start=True, stop=True)
                    bcs = work.tile([128, TT], F32, name=f"bcs{ti}_{p}", tag="bcs",
                                    bufs=3)
                    dve.reciprocal_approx_fast(bcs[:, :], bc[:, :])
                    dve.tensor_tensor(_r(avn[0:DH, p, :]), av_pair[0][0:DH, :],
                                      bcs[0:DH, :], op=OP.mult)
                    dve.tensor_tensor(_r(avn[DH:128, p, :]), av_pair[1][DH:128, :],
                                      bcs[DH:128, :], op=OP.mult)

                # output projection + bias + residual
                out_sb = work.tile([128, MC, TT], F32, name=f"o{ti}", tag="osb",
                                   bufs=2)
                for m in range(MC):
                    pso = ps1.tile([128, TT], F32, name=f"pso{ti}_{m}",
                                   tag=("psq" if m % 2 else "pso")
                                   if ti == NT - 1 else "pso", bufs=1)
                    for kc in range(MC):
                        pe.matmul(pso[:, :],
                                  _r(wo_sb[:, kc, m * 128:(m + 1) * 128]),
                                  _r(avn[:, kc, :]), start=(kc == 0),
                                  stop=(kc == MC - 1))
                    dve.scalar_tensor_tensor(out_sb[:, m, :], pso[:, :],
                                             vcol("bo", m), xall[:, m, tsl],
                                             op0=OP.add, op1=OP.add)
                    sync.dma_start(out_view[:, m, tsl], out_sb[:, m, :])


_CACHE = {}


def _build():
    if "nc" in _CACHE:
        return _CACHE["nc"]
    nc = bacc.Bacc("TRN2", target_bir_lowering=False, debug=False,
                   num_devices=NCORES)
    d = {}
    d["x"] = nc.dram_tensor("x", [C, L], F32, kind="ExternalInput").ap()
    d["ctx"] = nc.dram_tensor("ctx", [CC, S], F32, kind="ExternalInput").ap()
    d["wqT"] = nc.dram_tensor("wqT", [C, INNER], F32, kind="ExternalInput").ap()
    d["wkvT"] = nc.dram_tensor("wkvT", [CC, 2 * INNER], F32,
                               kind="ExternalInput").ap()
    d["woT"] = nc.dram_tensor("woT", [INNER, C], F32, kind="ExternalInput").ap()
    d["vecs"] = nc.dram_tensor("vecs", [128, VCOLS], F32,
                               kind="ExternalInput").ap()
    d["indall"] = nc.dram_tensor("indall", [(CCHUNK + XC) * 128, G], F32,
                                 kind="ExternalInput").ap()
    d["indTall"] = nc.dram_tensor("indTall", [G, CC + C + INNER], F32,
                                  kind="ExternalInput").ap()
    d["ones128"] = nc.dram_tensor("ones128", [128, 128], F32,
                                  kind="ExternalInput").ap()
    d["vaug_pat"] = nc.dram_tensor("vaug_pat", [128, NH * 128], F32,
                                   kind="ExternalInput").ap()
    d["patd"] = nc.dram_tensor("patd", [128, 128], F32,
                               kind="ExternalInput").ap()
    d["zeros1k"] = nc.dram_tensor("zeros1k", [128, 2 * TT], F32,
                                  kind="ExternalInput").ap()
    d["out"] = nc.dram_tensor("out", [C, L], F32, kind="ExternalOutput").ap()

    with tile.TileContext(nc) as tc:
        _emit(nc, tc, d)
    nc.compile()
    _CACHE["nc"] = nc
    return nc


def _host_inputs(inputs):
    f = np.float32
    x = np.ascontiguousarray(inputs["x"].reshape(B, C, L), dtype=f)
    ctx = np.ascontiguousarray(inputs["context"], dtype=f)
    wq = np.asarray(inputs["wq"], dtype=f)
    wkv = np.asarray(inputs["wkv"], dtype=f)
    wo = np.asarray(inputs["wo"], dtype=f)
    bkv = np.asarray(inputs["bkv"], dtype=f)

    ind_x = np.zeros((C, G), f)
    ind_x[np.arange(C), np.arange(C) // (C // G)] = 1.0
    ind_c = np.zeros((CC, G), f)
    ind_c[np.arange(CC), np.arange(CC) // (CC // G)] = 1.0
    # indall rows: ctx chunks first, then x chunks
    indall = np.ascontiguousarray(np.concatenate([ind_c, ind_x], axis=0))
    # indTall: [indT_c (CC) | indT_x (C) | row0 = bkv_v (INNER)]
    indTall = np.zeros((G, CC + C + INNER), f)
    indTall[:, :CC] = ind_c.T
    indTall[:, CC:CC + C] = ind_x.T
    indTall[0, CC + C:] = bkv[INNER:]

    vaug_pat = np.zeros((128, NH * 128), f)
    for h in range(NH):
        vaug_pat[:, h * 128 + (DH if h % 2 == 0 else 0)] = 1.0
    # denominator broadcast: row 64 (even head) -> out 0..63,
    # row 0 (odd head) -> out 64..127
    patd = np.zeros((128, 128), f)
    patd[DH, 0:DH] = 1.0
    patd[0, DH:128] = 1.0

    def cols(vec, n):
        return np.asarray(vec, dtype=f).reshape(n, 128).T  # [128, n]

    vecs = np.zeros((128, VCOLS), f)
    vecs[:, VOFF["bq"]:VOFF["bq"] + 4] = cols(inputs["bq"], 4)
    vecs[:, VOFF["bkvk"]:VOFF["bkvk"] + 4] = cols(bkv[:INNER], 4)
    vecs[:, VOFF["bo"]:VOFF["bo"] + 4] = cols(inputs["bo"], 4)
    vecs[:, VOFF["gxg"]:VOFF["gxg"] + 4] = cols(inputs["gnx_g"], 4)
    vecs[:, VOFF["gxb"]:VOFF["gxb"] + 4] = cols(inputs["gnx_b"], 4)
    vecs[:, VOFF["gcg"]:VOFF["gcg"] + 6] = cols(inputs["gnc_g"], 6)
    vecs[:, VOFF["gcb"]:VOFF["gcb"] + 6] = cols(inputs["gnc_b"], 6)
    vecs[:, VOFF["eps"]] = EPS

    shared = {
        "wqT": np.ascontiguousarray(wq.T),
        "wkvT": np.ascontiguousarray(wkv.T),
        "woT": np.ascontiguousarray(wo.T),
        "indall": indall,
        "indTall": indTall,
        "ones128": np.ones((128, 128), f),
        "vaug_pat": vaug_pat,
        "patd": patd,
        "zeros1k": np.zeros((128, 2 * TT), f),
        "vecs": vecs,
    }
    return [{"x": x[i], "ctx": ctx[i], **shared} for i in range(NCORES)]


def run(inputs, **spmd_kwargs):
    nc = _build()
    in_maps = _host_inputs(inputs)
    res = run_bass_kernel_spmd(nc, in_maps, list(range(NCORES)), **spmd_kwargs)
    out = np.stack([res.results[i]["out"] for i in range(NCORES)])
    return out.reshape(B, C, 64, 64).astype(np.float32), res


def kernel(**inputs) -> np.ndarray:
    out, _ = run(inputs)
    return out



# revision 25
# speedup vs baseline: 1.1461x; 1.1461x over previous
"""Trainium2 Bass kernel: AttentionBlock (GroupNorm + cross-attention + residual).

Sharding: data-parallel over batch. b=8 maps 1:1 onto the 8 NeuronCores;
each core computes its whole batch item, no collectives.

v2: fp8e4m3 DoubleRow matmuls for all projections and attention (0.5
cycles/output-column, 2x contraction rows per instruction), with every
channel/head ordering pair-interleaved via host-side weight permutations.
x and ctx ship as bf16 from the host. GroupNorm(x) stays folded into the
Q weights (wq8 = fp8(wqT*A), bqe = bq + wq@B); x is quantized to fp8 on
the gpsimd engine tile-by-tile. Softmax denominators are gathered by
per-head indicator matmuls into one [8,TT] PSUM tile, reciprocated in a
single small DVE op, and broadcast across partitions with one fp32r
matmul per head-pair; the av normalize + residual/bias tail alternate
between DVE and gpsimd to keep the scalar engine (exp, the hard floor)
as the only saturated engine.
"""

import sys

import numpy as np

sys.path.insert(0, "/opt/trn_rl_repo")

import ml_dtypes

import concourse.bacc as bacc
import concourse.bass as bass
import concourse.mybir as mybir
import concourse.tile as tile
from concourse.bass_utils import run_bass_kernel_spmd

F32 = mybir.dt.float32
F32R = mybir.dt.float32r
BF16 = mybir.dt.bfloat16
F8 = mybir.dt.float8e4
AF = mybir.ActivationFunctionType
OP = mybir.AluOpType
DR = mybir.MatmulPerfMode.DoubleRow

B = 8
C = 512
L = 4096          # 64*64
CC = 768
S = 256
INNER = 512
NH = 8
DH = 64
G = 32
EPS = 1e-5
TT = 512          # t-tile
NT = L // TT      # 8
NCORES = 8
SCALE2 = 1.0 / DH

NXB = 4           # x channel blocks (kc2, j)
NCB = 6           # ctx channel blocks
NQB = 4           # q/k out blocks (mq2, j)

# packed per-partition vector columns
VOFF = {"bq": 0, "bkvk": 4, "bo": 8, "gxg": 12, "gxb": 16, "gcg": 20,
        "gcb": 26, "eps": 32}
VCOLS = 36


def _r(ap):
    return ap.bitcast(F32R)


def _emit(nc, tc, d):
    sync = nc.sync
    act = nc.scalar
    dve = nc.vector
    pe = nc.tensor
    gp = nc.gpsimd
    ds = bass.ds

    with tc.tile_pool(name="keep", bufs=1) as keep:
        # ---------------- persistent tiles ----------------
        xall = keep.tile([128, 2, 2, L], BF16, name="xall", tag="xall")
        x8 = keep.tile([128, 2, 2, L], F8, name="x8", tag="x8")
        wq8 = keep.tile([128, 2, 2, INNER], F8, name="wq8", tag="wq8")
        wo8 = keep.tile([128, 2, 2, C], F8, name="wo8", tag="wo8")
        k8 = keep.tile([128, 4, S], F8, name="k8", tag="k8")
        v8 = keep.tile([128, 2, NH, 128], F8, name="v8", tag="v8")
        ones8 = keep.tile([128, 2, NH, 36], F8, name="ones8", tag="ones8")
        pat2 = keep.tile([4, 2, 128], F32, name="pat2", tag="pat2")
        vecs = keep.tile([128, VCOLS], F32, name="vecs", tag="vecs")
        bqe = keep.tile([128, NQB], F32, name="bqe", tag="bqe")
        rowm = keep.tile([1, 128 + INNER], F32, name="rowm", tag="rowm")

        def vcol(nm, j=0):
            return vecs[:, VOFF[nm] + j:VOFF[nm] + j + 1]

        with tc.tile_pool(name="sb0", bufs=1) as sb0, \
             tc.tile_pool(name="ps0", bufs=1, space="PSUM") as ps0:

            ctx_sb = sb0.tile([128, 3, 2, S], BF16, name="ctx_sb", tag="ctx_sb")
            gnc8 = sb0.tile([128, 3, 2, S], F8, name="gnc8", tag="gnc8")
            indall = sb0.tile([128, NCB + NXB, G], F32, name="indall",
                              tag="indall")
            indTall = sb0.tile([G, (NCB + NXB) * 128], F32, name="indTall",
                               tag="indTall")
            wqT_sb = sb0.tile([128, 2, 2, INNER], F32, name="wqT_sb",
                              tag="wqT_sb")
            wkv8 = sb0.tile([128, 3, 2, 2 * INNER], F8, name="wkv8", tag="wkv8")

            # ---- DMA schedule ----
            sync.dma_start(ctx_sb[:, :, :, :], d["ctx"].rearrange(
                "p (a b s) -> p a b s", a=3, b=2))
            sync.dma_start(_r(indall[:, :, :]),
                           _r(d["indall"].rearrange("(j p) g -> p j g", p=128)))
            sync.dma_start(_r(indTall[:, :]), _r(d["indTall"][:, :]))
            sync.dma_start(vecs[:, :], d["vecs"][:, :])
            sync.dma_start(_r(rowm[:, :]), _r(d["rowm"][:, :]))
            xv = d["x"].rearrange("p (a b l) -> p a b l", a=2, b=2)
            for kc2 in range(2):
                for j in range(2):
                    sync.dma_start(xall[:, kc2, j, :], xv[:, kc2, j, :])
            sync.dma_start(_r(wqT_sb[:, :, :, :]), _r(d["wqT"].rearrange(
                "p (a b o) -> p a b o", a=2, b=2)))
            sync.dma_start(wkv8[:, :, :, :], d["wkv8"].rearrange(
                "p (a b o) -> p a b o", a=3, b=2))
            sync.dma_start(wo8[:, :, :, :], d["wo8"].rearrange(
                "p (a b o) -> p a b o", a=2, b=2))
            sync.dma_start(ones8[:, :, :, :], d["ones8"].rearrange(
                "p (a h r) -> p a h r", a=2, h=NH))
            sync.dma_start(_r(pat2[:, :, :]), _r(d["pat2"].rearrange(
                "r (q c) -> r q c", q=2)))

            gp.memset(v8[:, :, :, :], 0.0)

            ind_c = [indall[:, j, :] for j in range(NCB)]
            ind_x = [indall[:, NCB + m, :] for m in range(NXB)]

            def indT_c(j):
                return indTall[:, j * 128:(j + 1) * 128]

            def indT_x(m):
                return indTall[:, (NCB + m) * 128:(NCB + m + 1) * 128]

            ones_row = rowm[0:1, 0:128]
            bkvv_row = rowm[0:1, 128:128 + INNER]

            def chan_stats(src, nblks, nsub, sub, ind_tiles, inv_n, tagp):
                bns = []
                for jb in range(nblks):
                    bn = sb0.tile([128, nsub * 6], F32, name=f"bn{tagp}{jb}",
                                  tag=f"bn{tagp}{jb}")
                    bns.append(bn.rearrange("p (a q) -> p a q", q=6))
                for jb in range(nblks):
                    for a in range(nsub):
                        dve.bn_stats(bns[jb][:, a, :],
                                     src(jb)[:, a * sub:(a + 1) * sub])
                rhs_list = []
                for jb in range(nblks):
                    st = sb0.tile([128, 2], F32, name=f"st{tagp}{jb}",
                                  tag=f"st{tagp}{jb}")
                    dve.bn_aggr(st[:, :], bns[jb])
                    r2 = sb0.tile([128, 2], F32, name=f"r2{tagp}{jb}",
                                  tag=f"r2{tagp}{jb}")
                    dve.tensor_copy(_r(r2[:, 0:1]), st[:, 0:1])
                    dve.scalar_tensor_tensor(_r(r2[:, 1:2]), st[:, 0:1],
                                             st[:, 0:1], st[:, 1:2],
                                             op0=OP.mult, op1=OP.add)
                    rhs_list.append(r2)
                psg = ps0.tile([G, 2], F32, name=f"psg{tagp}", tag="misc",
                               bufs=2)
                for jb in range(nblks):
                    pe.matmul(psg[:, :], ind_tiles[jb], rhs_list[jb][:, :],
                              start=(jb == 0), stop=(jb == nblks - 1))
                gstat = sb0.tile([G, 2], F32, name=f"gstat{tagp}",
                                 tag=f"gstat{tagp}")
                act.mul(gstat[:, :], psg[:, :], inv_n)
                nvar = sb0.tile([G, 1], F32, name=f"nvar{tagp}",
                                tag=f"nvar{tagp}")
                dve.scalar_tensor_tensor(nvar[:, :], gstat[:, 0:1],
                                         gstat[:, 0:1], gstat[:, 1:2],
                                         op0=OP.mult, op1=OP.subtract)
                sd = sb0.tile([G, 1], F32, name=f"sd{tagp}", tag=f"sd{tagp}")
                act.activation(sd[:, :], nvar[:, :], AF.Sqrt, scale=-1.0,
                               bias=vcol("eps")[0:G, :])
                rstd = sb0.tile([G, 1], F32, name=f"rstd{tagp}",
                                tag=f"rstd{tagp}")
                dve.reciprocal(rstd[:, :], sd[:, :])
                er = sb0.tile([G, 2], F32, name=f"er{tagp}", tag=f"er{tagp}")
                dve.tensor_copy(_r(er[:, 0:1]), rstd[:, :])
                dve.tensor_copy(_r(er[:, 1:2]), gstat[:, 0:1])
                return er

            def expand_ab(er, indT, jb, gam, bet, tagp):
                pse = ps0.tile([128, 2], F32, name=f"pse{tagp}{jb}", tag="misc",
                               bufs=2)
                pe.matmul(pse[:, :], indT, er[:, :], start=True, stop=True)
                A = sb0.tile([128, 1], F32, name=f"A{tagp}{jb}",
                             tag=f"A{tagp}{jb}")
                dve.tensor_tensor(A[:, :], pse[:, 0:1], gam, op=OP.mult)
                Bt = sb0.tile([128, 1], F32, name=f"B{tagp}{jb}",
                              tag=f"B{tagp}{jb}")
                muA = sb0.tile([128, 1], F32, name=f"muA{tagp}{jb}",
                               tag=f"muA{tagp}", bufs=2)
                dve.tensor_tensor(muA[:, :], pse[:, 1:2], A[:, :], op=OP.mult)
                dve.tensor_tensor(_r(Bt[:, :]), bet, muA[:, :], op=OP.subtract)
                return A, Bt

            # ---- context groupnorm (materialized, fp8 out) ----
            er_c = chan_stats(lambda jb: ctx_sb[:, jb // 2, jb % 2, :], NCB, 1,
                              S, ind_c, 1.0 / (CC // G), "c")
            for jb in range(NCB):
                A, Bt = expand_ab(er_c, indT_c(jb), jb, vcol("gcg", jb),
                                  vcol("gcb", jb), "c")
                dve.tensor_scalar(gnc8[:, jb // 2, jb % 2, :],
                                  ctx_sb[:, jb // 2, jb % 2, :], A[:, :],
                                  Bt[:, :], op0=OP.mult, op1=OP.add)

            # ---- x8 quantize (gpsimd): first two tiles here, the rest are
            # emitted inside the t-loop two tiles ahead ----
            for ti in range(2):
                tsl = ds(ti * TT, TT)
                gp.tensor_copy(x8[:, :, :, tsl], xall[:, :, :, tsl])

            # ---- x stats -> fold into wq8 / bqe ----
            er_x = chan_stats(lambda jb: xall[:, jb // 2, jb % 2, :], NXB, 8,
                              512, ind_x, 1.0 / (C // G), "x")
            Bx = []
            for jb in range(NXB):
                A, Bt = expand_ab(er_x, indT_x(jb), jb, vcol("gxg", jb),
                                  vcol("gxb", jb), "x")
                act.activation(wq8[:, jb // 2, jb % 2, :],
                               wqT_sb[:, jb // 2, jb % 2, :], AF.Copy,
                               scale=A[:, :])
                Bx.append(Bt)
            for b in range(NQB):
                psb = ps0.tile([128, 1], F32, name=f"psb{b}", tag="misc",
                               bufs=2)
                for jb in range(NXB):
                    pe.matmul(psb[:, :],
                              wqT_sb[:, jb // 2, jb % 2,
                                     b * 128:(b + 1) * 128],
                              Bx[jb][:, :], start=(jb == 0),
                              stop=(jb == NXB - 1))
                dve.tensor_tensor(bqe[:, b:b + 1], psb[:, :], vcol("bq", b),
                                  op=OP.add)

            # ---- K and V^T projections (fp8 DoubleRow) ----
            psk = [ps0.tile([128, S], F32, name=f"psk{b}", tag=f"psk{b}")
                   for b in range(NQB)]
            psv = [ps0.tile([128, INNER], F32, name=f"psv{sc}", tag=f"psv{sc}")
                   for sc in range(2)]
            for kc2 in range(3):
                for b in range(NQB):
                    pe.matmul(psk[b][:, :],
                              wkv8[:, kc2, :, b * 128:(b + 1) * 128],
                              gnc8[:, kc2, :, :], start=(kc2 == 0),
                              stop=(kc2 == 2), perf_mode=DR)
                for sc in range(2):
                    pe.matmul(psv[sc][:, :],
                              gnc8[:, kc2, :, sc * 128:(sc + 1) * 128],
                              wkv8[:, kc2, :, INNER:2 * INNER],
                              start=(kc2 == 0), stop=False, perf_mode=DR)
            for sc in range(2):
                pe.matmul(psv[sc][:, :], _r(ones_row), _r(bkvv_row),
                          start=False, stop=True)
            for b in range(NQB):
                act.activation(k8[:, b, :], psk[b][:, :],
                               AF.Identity, bias=vcol("bkvk", b))
            for sc in range(2):
                for par in range(2):
                    dve.tensor_copy(v8[:, sc, par::2, par * DH:(par + 1) * DH],
                                    psv[sc].rearrange("p (h c) -> p h c",
                                                      c=DH)[:, par::2, :])

        # ---------------- t-loop: Q / attention / out-proj ----------------
        with tc.tile_pool(name="work", bufs=1) as work, \
             tc.tile_pool(name="ps1", bufs=1, space="PSUM") as ps1:
            out_view = d["out"].rearrange("(a p b) l -> p a b l", p=128, b=2)
            # Flat cross-tile software pipeline. PSUM (8 banks): psd ring
            # 2x[128,2,TT] (4) + big ring 3x[128,TT] (3, shared by av, bc,
            # psq and pso) + psg32 (1). Heads processed as two quads; the
            # denominator gather lands at psg32 offsets 0/64 (the only legal
            # matmul output base partitions besides 32).
            q8_t = {}
            E_t = {}
            psg_t = {}
            avn_t = {}

            def emit_q(t, b):
                tsl = ds(t * TT, TT)
                if b == 0:
                    q8_t[t] = work.tile([128, 4, TT], F8, name=f"q8_{t}",
                                        tag="q8", bufs=2)
                psq = ps1.tile([128, TT], F32, name=f"psq{t}_{b}", tag="big",
                               bufs=3)
                for kc2 in range(2):
                    pe.matmul(psq[:, :], wq8[:, kc2, :, b * 128:(b + 1) * 128],
                              x8[:, kc2, :, tsl], start=(kc2 == 0),
                              stop=(kc2 == 1), perf_mode=DR)
                if b % 2 == 0:
                    dve.tensor_scalar_add(q8_t[t][:, b, :], psq[:, :],
                                          bqe[:, b:b + 1])
                else:
                    act.activation(q8_t[t][:, b, :], psq[:, :], AF.Identity,
                                   bias=bqe[:, b:b + 1])

            def dots_exp(t, h):
                psd = ps1.tile([128, 2, TT], F32, name=f"psd{t}_{h}",
                               tag="psd", bufs=2)
                h4 = (h % 2) * DH
                q8 = q8_t[t]
                for sc in range(2):
                    pe.matmul(psd[:, sc, :],
                              k8[h4:h4 + DH, h // 2,
                                 sc * 128:(sc + 1) * 128],
                              q8[h4:h4 + DH, h // 2, :],
                              start=True, stop=True)
                E8 = work.tile([128, 2, TT], F8, name=f"E{t}_{h}", tag="E",
                               bufs=4)
                act.activation(E8[:, :, :], psd[:, :, :], AF.Exp,
                               scale=SCALE2)
                E_t[(t, h)] = E8

            def av_mm(t, p2):
                """AV + denominator-gather matmuls for head pair p2."""
                if p2 == 0:
                    psg_t[t] = ps1.tile([128, TT], F32, name=f"psg{t}",
                                        tag="psg32", bufs=1)
                    avn_t[t] = work.tile([128, 4, TT], F8, name=f"avn{t}",
                                         tag="avn", bufs=2)
                psg32 = psg_t[t]
                av = ps1.tile([128, TT], F32, name=f"av{t}_{p2}", tag="big",
                              bufs=3)
                qd = p2 // 2
                for r2 in range(2):
                    h = 2 * p2 + r2
                    E8 = E_t.pop((t, h))
                    pe.matmul(av[:, :], v8[:, :, h, :], E8[:, :, :],
                              start=(r2 == 0), stop=(r2 == 1), perf_mode=DR)
                    gsl = ones8[:, :, h, 0:4] if qd == 0 else ones8[:, :, h, :]
                    nr = 4 if qd == 0 else 36
                    pe.matmul(psg32[0:nr, :], gsl, E8[:, :, :],
                              start=(h % 4 == 0), stop=(h % 4 == 3),
                              perf_mode=DR)
                return av

            def norm_pair(t, p2, av, rcpQ):
                """broadcast 1/denom for pair p2 and normalize its av."""
                bc = ps1.tile([128, TT], F32, name=f"bc{t}_{p2}", tag="big",
                              bufs=3)
                pe.matmul(bc[:, :], _r(pat2[:, p2 % 2, :]), _r(rcpQ[:, :]),
                          start=True, stop=True)
                bcs = work.tile([128, TT], F32, name=f"bcs{t}_{p2}", tag="bcs",
                                bufs=2)
                if p2 % 2 == 0:
                    act.copy(bcs[:, :], bc[:, :])
                else:
                    dve.tensor_copy(bcs[:, :], bc[:, :])
                dve.tensor_tensor(avn_t[t][:, p2, :], av[:, :], bcs[:, :],
                                  op=OP.mult)

            def rcp_quad(t, qd):
                rcpQ = work.tile([4, TT], F32, name=f"rcp{t}_{qd}", tag="rcp",
                                 bufs=2)
                with nc.allow_low_precision(reason="f32r bitcast, still 32-bit"):
                    dve.reciprocal(_r(rcpQ[:, :]),
                                   psg_t[t][32 * qd:32 * qd + 4, :])
                return rcpQ

            out_t = {}

            def emit_o(t, m):
                tsl = ds(t * TT, TT)
                if m == 0:
                    out_t[t] = work.tile([128, 2, 2, TT], F32, name=f"o{t}",
                                         tag="osb", bufs=2)
                pso = ps1.tile([128, TT], F32, name=f"pso{t}_{m}", tag="big",
                               bufs=3)
                for kq2 in range(2):
                    pe.matmul(pso[:, :], wo8[:, kq2, :, m * 128:(m + 1) * 128],
                              avn_t[t][:, 2 * kq2:2 * kq2 + 2, :],
                              start=(kq2 == 0), stop=(kq2 == 1), perf_mode=DR)
                eng = dve
                eng.scalar_tensor_tensor(out_t[t][:, m // 2, m % 2, :],
                                         pso[:, :], vcol("bo", m),
                                         xall[:, m // 2, m % 2, tsl],
                                         op0=OP.add, op1=OP.add)
                if m == 3:
                    ot = out_t.pop(t)
                    for kc2 in range(2):
                        sync.dma_start(out_view[:, kc2, :, tsl],
                                       ot[:, kc2, :, :])
                    avn_t.pop(t)

            # prologue
            for b in range(NQB):
                emit_q(0, b)
            dots_exp(0, 0)
            dots_exp(0, 1)
            for t in range(NT):
                for qd in range(2):
                    pa, pb = 2 * qd, 2 * qd + 1
                    dots_exp(t, 4 * qd + 2)
                    dots_exp(t, 4 * qd + 3)
                    av_a = av_mm(t, pa)
                    if qd == 0:
                        dots_exp(t, 4)
                        dots_exp(t, 5)
                    elif t + 1 < NT:
                        emit_q(t + 1, 0)
                        emit_q(t + 1, 1)
                        dots_exp(t + 1, 0)
                        dots_exp(t + 1, 1)
                    av_b = av_mm(t, pb)
                    rcpQ = rcp_quad(t, qd)
                    norm_pair(t, pa, av_a, rcpQ)
                    norm_pair(t, pb, av_b, rcpQ)
                    if t > 0:
                        emit_o(t - 1, 2 * qd)
                        emit_o(t - 1, 2 * qd + 1)
                if t + 1 < NT:
                    if t + 2 < NT:
                        gp.tensor_copy(x8[:, :, :, ds((t + 2) * TT, TT)],
                                       xall[:, :, :, ds((t + 2) * TT, TT)])
                    emit_q(t + 1, 2)
                    emit_q(t + 1, 3)
            for m in range(4):
                emit_o(NT - 1, m)


_CACHE = {}


def _build():
    if "nc" in _CACHE:
        return _CACHE["nc"]
    nc = bacc.Bacc("TRN2", target_bir_lowering=False, debug=False,
                   num_devices=NCORES)
    d = {}
    d["x"] = nc.dram_tensor("x", [128, 4 * L], BF16, kind="ExternalInput").ap()
    d["ctx"] = nc.dram_tensor("ctx", [128, 6 * S], BF16,
                              kind="ExternalInput").ap()
    d["wqT"] = nc.dram_tensor("wqT", [128, 4 * INNER], F32,
                              kind="ExternalInput").ap()
    d["wkv8"] = nc.dram_tensor("wkv8", [128, 6 * INNER * 2], F8,
                               kind="ExternalInput").ap()
    d["wo8"] = nc.dram_tensor("wo8", [128, 4 * C], F8,
                              kind="ExternalInput").ap()
    d["vecs"] = nc.dram_tensor("vecs", [128, VCOLS], F32,
                               kind="ExternalInput").ap()
    d["indall"] = nc.dram_tensor("indall", [(NCB + NXB) * 128, G], F32,
                                 kind="ExternalInput").ap()
    d["indTall"] = nc.dram_tensor("indTall", [G, (NCB + NXB) * 128], F32,
                                  kind="ExternalInput").ap()
    d["ones8"] = nc.dram_tensor("ones8", [128, 2 * NH * 36], F8,
                                kind="ExternalInput").ap()
    d["pat2"] = nc.dram_tensor("pat2", [4, 2 * 128], F32,
                               kind="ExternalInput").ap()
    d["rowm"] = nc.dram_tensor("rowm", [1, 128 + INNER], F32,
                               kind="ExternalInput").ap()
    d["out"] = nc.dram_tensor("out", [C, L], F32, kind="ExternalOutput").ap()

    with tile.TileContext(nc) as tc:
        _emit(nc, tc, d)
    nc.compile()
    _CACHE["nc"] = nc
    return nc


# ---- host-side orderings ----
def _x_chan(kc2, p, j):
    return kc2 * 256 + 2 * p + j


def _q_chan(b, c):
    mq2, jq = b // 2, b % 2
    return (mq2 * 4 + c // 32) * 64 + (c % 32) * 2 + jq


def _host_inputs(inputs):
    f = np.float32
    bf = ml_dtypes.bfloat16
    f8 = ml_dtypes.float8_e4m3fn

    x = np.asarray(inputs["x"], dtype=f).reshape(B, C, L)
    ctx = np.asarray(inputs["context"], dtype=f)
    wq = np.asarray(inputs["wq"], dtype=f)
    wkv = np.asarray(inputs["wkv"], dtype=f)
    wo = np.asarray(inputs["wo"], dtype=f)
    bkv = np.asarray(inputs["bkv"], dtype=f)

    p_ = np.arange(128)
    # x/out channel order: channel(p; kc2, j) = kc2*256 + 2p + j
    xch = np.empty((2, 128, 2), np.int64)
    for kc2 in range(2):
        for j in range(2):
            xch[kc2, :, j] = _x_chan(kc2, p_, j)
    xperm = xch.transpose(1, 0, 2).reshape(128, 4)     # [p, (kc2,j)]
    # ctx channel order
    cch = np.empty((3, 128, 2), np.int64)
    for kc2 in range(3):
        for j in range(2):
            cch[kc2, :, j] = kc2 * 256 + 2 * p_ + j
    cperm = cch.transpose(1, 0, 2).reshape(128, 6)     # [p, (kc2,j)]
    # q/k inner order: natural (head h at rows (h%2)*64 of block h//2)
    qcols = np.arange(NQB * 128).reshape(NQB, 128)
    qorder = qcols.reshape(-1)

    # x_dev [128, (kc2,j,L)] bf16
    x_dev = np.empty((B, 128, 2, 2, L), bf)
    for kc2 in range(2):
        for j in range(2):
            x_dev[:, :, kc2, j, :] = x[:, xch[kc2, :, j], :].astype(bf)
    x_dev = x_dev.reshape(B, 128, 4 * L)

    # ctx_dev [128, (kc2,j,S)] bf16
    ctx_dev = np.empty((B, 128, 3, 2, S), bf)
    for kc2 in range(3):
        for j in range(2):
            ctx_dev[:, :, kc2, j, :] = ctx[:, cch[kc2, :, j], :].astype(bf)
    ctx_dev = ctx_dev.reshape(B, 128, 6 * S)

    # wqT_dev [128, (kc2,j,512cols)] f32 : wq[qorder(col), xchan(p,kc2,j)]
    wqT_dev = np.empty((128, 2, 2, INNER), f)
    for kc2 in range(2):
        for j in range(2):
            wqT_dev[:, kc2, j, :] = wq[np.ix_(qorder, xch[kc2, :, j])].T
    wqT_dev = wqT_dev.reshape(128, 4 * INNER)

    # wkv8_dev [128, (kc2,j, k512 | v512)] fp8
    wkv8_dev = np.empty((128, 3, 2, 2 * INNER), f)
    vorder = np.arange(INNER) + INNER        # natural v rows of wkv
    for kc2 in range(3):
        for j in range(2):
            cc = cch[kc2, :, j]
            wkv8_dev[:, kc2, j, :INNER] = wkv[np.ix_(qorder, cc)].T
            wkv8_dev[:, kc2, j, INNER:] = wkv[np.ix_(vorder, cc)].T
    wkv8_dev = wkv8_dev.reshape(128, 12 * INNER).astype(f8)

    # wo8_dev [128, (kq2,jq, 512cols)] fp8 : wo[outchan(col), (2kq2+jq)*128+p]
    outcols = np.empty((4, 128), np.int64)
    for bo in range(4):
        outcols[bo] = _x_chan(bo // 2, np.arange(128), bo % 2)
    wo8_dev = np.empty((128, 2, 2, C), f)
    for kq2 in range(2):
        for jq in range(2):
            inner_idx = (2 * kq2 + jq) * 128 + p_
            wo8_dev[:, kq2, jq, :] = wo[np.ix_(outcols.reshape(-1),
                                               inner_idx)].T
    wo8_dev = wo8_dev.reshape(128, 4 * C).astype(f8)

    # indicator matrices (permuted orders)
    ind_x = np.zeros((NXB, 128, G), f)
    indT_x = np.zeros((NXB, G, 128), f)
    for blk in range(NXB):
        g = xch[blk // 2, :, blk % 2] // (C // G)
        ind_x[blk, p_, g] = 1.0
        indT_x[blk, g, p_] = 1.0
    ind_c = np.zeros((NCB, 128, G), f)
    indT_c = np.zeros((NCB, G, 128), f)
    for blk in range(NCB):
        g = cch[blk // 2, :, blk % 2] // (CC // G)
        ind_c[blk, p_, g] = 1.0
        indT_c[blk, g, p_] = 1.0
    indall = np.concatenate([ind_c, ind_x], axis=0).reshape(-1, G)
    indall = np.ascontiguousarray(indall)
    indTall = np.ascontiguousarray(
        np.concatenate([indT_c, indT_x], axis=0).transpose(1, 0, 2)
        .reshape(G, -1))

    # gather patterns: quad 0 heads hit rows 0-3, quad 1 rows 32-35
    ones8 = np.zeros((128, 2, NH, 36), f)
    for h in range(NH):
        ones8[:, :, h, (0 if h < 4 else 32) + h % 4] = 1.0
    ones8 = ones8.reshape(128, 2 * NH * 36).astype(f8)

    # pat2[r4, pq, c] = (r4 == 2*pq + (c >= 64)): pair rows -> 64|64
    pat2 = np.zeros((4, 2, 128), f)
    for pq in range(2):
        pat2[2 * pq, pq, 0:DH] = 1.0
        pat2[2 * pq + 1, pq, DH:128] = 1.0
    pat2 = pat2.reshape(4, 2 * 128)

    def cols_perm(vec, order):
        return np.asarray(vec, dtype=f)[order]          # [128, n]

    vecs = np.zeros((128, VCOLS), f)
    vecs[:, VOFF["bq"]:VOFF["bq"] + 4] = cols_perm(inputs["bq"], qcols.T)
    vecs[:, VOFF["bkvk"]:VOFF["bkvk"] + 4] = cols_perm(bkv[:INNER], qcols.T)
    vecs[:, VOFF["bo"]:VOFF["bo"] + 4] = cols_perm(inputs["bo"],
                                                   outcols.T)
    vecs[:, VOFF["gxg"]:VOFF["gxg"] + 4] = cols_perm(inputs["gnx_g"], xperm)
    vecs[:, VOFF["gxb"]:VOFF["gxb"] + 4] = cols_perm(inputs["gnx_b"], xperm)
    vecs[:, VOFF["gcg"]:VOFF["gcg"] + 6] = cols_perm(inputs["gnc_g"], cperm)
    vecs[:, VOFF["gcb"]:VOFF["gcb"] + 6] = cols_perm(inputs["gnc_b"], cperm)
    vecs[:, VOFF["eps"]] = EPS

    rowm = np.zeros((1, 128 + INNER), f)
    rowm[0, :128] = 1.0
    rowm[0, 128:] = bkv[INNER:]

    shared = {
        "wqT": wqT_dev,
        "wkv8": wkv8_dev,
        "wo8": wo8_dev,
        "indall": indall,
        "indTall": indTall,
        "ones8": ones8,
        "pat2": pat2,
        "rowm": rowm,
        "vecs": vecs,
    }
    return [{"x": x_dev[i], "ctx": ctx_dev[i], **shared} for i in range(NCORES)]


def run(inputs, **spmd_kwargs):
    nc = _build()
    in_maps = _host_inputs(inputs)
    res = run_bass_kernel_spmd(nc, in_maps, list(range(NCORES)), **spmd_kwargs)
    out = np.stack([np.asarray(res.results[i]["out"], dtype=np.float32)
                    for i in range(NCORES)])
    return out.reshape(B, C, 64, 64), res


def kernel(**inputs) -> np.ndarray:
    out, _ = run(inputs)
    return out


# revision 53
# speedup vs baseline: 1.3937x; 1.2160x over previous
"""Trainium2 Bass kernel: AttentionBlock (GroupNorm + cross-attention + residual).

Sharding: data-parallel over batch. b=8 maps 1:1 onto the 8 NeuronCores;
each core computes its whole batch item, no collectives.

Design (baseline 188.8us -> ~135.6us modeled):
  - fp8e4m3 DoubleRow matmuls (0.5 cycles/output-column, 2x contraction
    rows per instruction) for the Q/K/V/out projections and AV; dots run
    plain fp8 (per-head K=64 at partition offsets 0/64 - DR would need
    offset 96, which the ISA rejects). Channel pairings for every DR
    contraction are pre-interleaved in the host-side weight layouts.
  - x and ctx ship as bf16 from the host (half the DMA, exact enough for
    the residual); fp8 operands x8/q8/k8/v8/E8/avn8 feed the PE.
  - GroupNorm(x) is folded into the Q weights (wq8 = fp8(wqT*A), bqe =
    bq + wq@B). Group stats: blocks 0-1 of x via fused ACT passes
    (Copy/Square with accum_out - the Copy pass doubles as the x8 cast),
    blocks 2-3 via DVE bn_stats chasing split DMAs; gpsimd casts x8 for
    blocks 2-3 tile-by-tile ahead of the Q projection.
  - Softmax: exp on ACT in [128, 2, TT] batches (the hard floor: the
    scalar engine is the only exp engine, NH*S*L/128 columns ~ 55us).
    Denominators ride per-head indicator matmuls accumulated per quad
    into psum rows {0,1,32,33}, one f32r reciprocal per quad, partition
    broadcast by a tiny fp32r matmul per pair, materialized to SBUF on
    ACT, then a single DVE multiply normalizes each av pair.
  - Flat cross-tile software pipeline: tile t's attention overlaps tile
    t-1's out-projection/residual and tile t+1's Q projection; PSUM is
    exactly 8 banks (dots ring 2x[128,2,TT], av ring 2, a shared bank
    for the denominator/broadcast psums, one for psq/pso).
"""

import sys

import numpy as np

sys.path.insert(0, "/opt/trn_rl_repo")

import ml_dtypes

import concourse.bacc as bacc
import concourse.bass as bass
import concourse.mybir as mybir
import concourse.tile as tile
from concourse.bass_utils import run_bass_kernel_spmd

F32 = mybir.dt.float32
F32R = mybir.dt.float32r
BF16 = mybir.dt.bfloat16
F8 = mybir.dt.float8e4
AF = mybir.ActivationFunctionType
OP = mybir.AluOpType
DR = mybir.MatmulPerfMode.DoubleRow

B = 8
C = 512
L = 4096          # 64*64
CC = 768
S = 256
INNER = 512
NH = 8
DH = 64
G = 32
EPS = 1e-5
TT = 512          # t-tile
NT = L // TT      # 8
NCORES = 8
SCALE2 = 1.0 / DH

NXB = 4           # x channel blocks (kc2, j)
NCB = 6           # ctx channel blocks
NQB = 4           # q/k out blocks (mq2, j)

# packed per-partition vector columns
VOFF = {"bq": 0, "bkvk": 4, "bo": 8, "gxg": 12, "gxb": 16, "gcg": 20,
        "gcb": 26, "eps": 32}
VCOLS = 36


def _r(ap):
    return ap.bitcast(F32R)


def _emit(nc, tc, d):
    sync = nc.sync
    act = nc.scalar
    dve = nc.vector
    pe = nc.tensor
    gp = nc.gpsimd
    ds = bass.ds

    with tc.tile_pool(name="keep", bufs=1) as keep:
        # ---------------- persistent tiles ----------------
        xall = keep.tile([128, 2, 2, L], BF16, name="xall", tag="xall")
        x8 = keep.tile([128, 2, 2, L], F8, name="x8", tag="x8")
        wq8 = keep.tile([128, 2, 2, INNER], F8, name="wq8", tag="wq8")
        wo8 = keep.tile([128, 2, 2, C], F8, name="wo8", tag="wo8")
        k8 = keep.tile([128, 4, S], F8, name="k8", tag="k8")
        v8 = keep.tile([128, 2, NH, 128], F8, name="v8", tag="v8")
        ones8 = keep.tile([128, 2, NH, 36], F8, name="ones8", tag="ones8")
        patP = keep.tile([36, 2, 128], F32, name="patP", tag="patP")
        vecs = keep.tile([128, VCOLS], F32, name="vecs", tag="vecs")
        bqe = keep.tile([128, NQB], F32, name="bqe", tag="bqe")
        rowm = keep.tile([1, 128 + INNER], F32, name="rowm", tag="rowm")

        def vcol(nm, j=0):
            return vecs[:, VOFF[nm] + j:VOFF[nm] + j + 1]

        with tc.tile_pool(name="sb0", bufs=1) as sb0, \
             tc.tile_pool(name="ps0", bufs=1, space="PSUM") as ps0:

            ctx_sb = sb0.tile([128, 3, 2, S], BF16, name="ctx_sb", tag="ctx_sb")
            gnc8 = sb0.tile([128, 3, 2, S], F8, name="gnc8", tag="gnc8")
            indall = sb0.tile([128, NCB + NXB, G], F32, name="indall",
                              tag="indall")
            indTall = sb0.tile([G, (NCB + NXB) * 128], F32, name="indTall",
                               tag="indTall")
            wqT_sb = sb0.tile([128, 2, 2, INNER], F32, name="wqT_sb",
                              tag="wqT_sb")
            wkv8 = sb0.tile([128, 3, 2, 2 * INNER], F8, name="wkv8", tag="wkv8")

            # ---- DMA schedule: x first (the stats chain gates phase A) ----
            xv = d["x"].rearrange("p (a b l) -> p a b l", a=2, b=2)
            for kc2 in range(2):
                for j in range(2):
                    if kc2 == 0:
                        sync.dma_start(xall[:, kc2, j, :], xv[:, kc2, j, :])
                    else:
                        for hf in range(2):
                            hsl = ds(hf * (L // 2), L // 2)
                            sync.dma_start(xall[:, kc2, j, hsl],
                                           xv[:, kc2, j, hsl])
            sync.dma_start(ctx_sb[:, :, :, :], d["ctx"].rearrange(
                "p (a b s) -> p a b s", a=3, b=2))
            sync.dma_start(_r(indall[:, :, :]),
                           _r(d["indall"].rearrange("(j p) g -> p j g", p=128)))
            sync.dma_start(_r(indTall[:, :]), _r(d["indTall"][:, :]))
            sync.dma_start(vecs[:, :], d["vecs"][:, :])
            sync.dma_start(_r(rowm[:, :]), _r(d["rowm"][:, :]))
            sync.dma_start(_r(wqT_sb[:, :, :, :]), _r(d["wqT"].rearrange(
                "p (a b o) -> p a b o", a=2, b=2)))
            sync.dma_start(wkv8[:, :, :, :], d["wkv8"].rearrange(
                "p (a b o) -> p a b o", a=3, b=2))
            sync.dma_start(wo8[:, :, :, :], d["wo8"].rearrange(
                "p (a b o) -> p a b o", a=2, b=2))
            sync.dma_start(ones8[:, :, :, :], d["ones8"].rearrange(
                "p (a h r) -> p a h r", a=2, h=NH))
            sync.dma_start(_r(patP[:, :, :]), _r(d["patP"].rearrange(
                "r (q c) -> r q c", q=2)))

            gp.memset(v8[:, :, :, :], 0.0)

            ind_c = [indall[:, j, :] for j in range(NCB)]
            ind_x = [indall[:, NCB + m, :] for m in range(NXB)]

            def indT_c(j):
                return indTall[:, j * 128:(j + 1) * 128]

            def indT_x(m):
                return indTall[:, (NCB + m) * 128:(NCB + m + 1) * 128]

            ones_row = rowm[0:1, 0:128]
            bkvv_row = rowm[0:1, 128:128 + INNER]

            def chan_stats(src, nblks, nsub, sub, ind_tiles, inv_n, tagp,
                           pre_rhs=()):
                bns = []
                for jb in range(nblks):
                    bn = sb0.tile([128, nsub * 6], F32, name=f"bn{tagp}{jb}",
                                  tag=f"bn{tagp}{jb}")
                    bns.append(bn.rearrange("p (a q) -> p a q", q=6))
                for jb in range(nblks):
                    for a in range(nsub):
                        dve.bn_stats(bns[jb][:, a, :],
                                     src(jb)[:, a * sub:(a + 1) * sub])
                rhs_list = []
                for jb in range(nblks):
                    st = sb0.tile([128, 2], F32, name=f"st{tagp}{jb}",
                                  tag=f"st{tagp}{jb}")
                    dve.bn_aggr(st[:, :], bns[jb])
                    r2 = sb0.tile([128, 2], F32, name=f"r2{tagp}{jb}",
                                  tag=f"r2{tagp}{jb}")
                    dve.tensor_copy(_r(r2[:, 0:1]), st[:, 0:1])
                    dve.scalar_tensor_tensor(_r(r2[:, 1:2]), st[:, 0:1],
                                             st[:, 0:1], st[:, 1:2],
                                             op0=OP.mult, op1=OP.add)
                    rhs_list.append(r2)
                psg = ps0.tile([G, 2], F32, name=f"psg{tagp}", tag="misc",
                               bufs=2)
                pairs = [(ind, mk()) for ind, mk in pre_rhs]
                pairs += [(ind_tiles[jb], rhs_list[jb]) for jb in range(nblks)]
                for i, (ind, r2) in enumerate(pairs):
                    pe.matmul(psg[:, :], ind, r2[:, :],
                              start=(i == 0), stop=(i == len(pairs) - 1))
                gstat = sb0.tile([G, 2], F32, name=f"gstat{tagp}",
                                 tag=f"gstat{tagp}")
                act.mul(gstat[:, :], psg[:, :], inv_n)
                nvar = sb0.tile([G, 1], F32, name=f"nvar{tagp}",
                                tag=f"nvar{tagp}")
                dve.scalar_tensor_tensor(nvar[:, :], gstat[:, 0:1],
                                         gstat[:, 0:1], gstat[:, 1:2],
                                         op0=OP.mult, op1=OP.subtract)
                sd = sb0.tile([G, 1], F32, name=f"sd{tagp}", tag=f"sd{tagp}")
                act.activation(sd[:, :], nvar[:, :], AF.Sqrt, scale=-1.0,
                               bias=vcol("eps")[0:G, :])
                er = sb0.tile([G, 2], F32, name=f"er{tagp}", tag=f"er{tagp}")
                with nc.allow_low_precision(reason="f32r bitcast is 32-bit"):
                    dve.reciprocal(_r(er[:, 0:1]), sd[:, :])
                dve.tensor_copy(_r(er[:, 1:2]), gstat[:, 0:1])
                return er

            def expand_ab(er, indT, jb, gam, bet, tagp):
                pse = ps0.tile([128, 2], F32, name=f"pse{tagp}{jb}", tag="misc",
                               bufs=2)
                pe.matmul(pse[:, :], indT, er[:, :], start=True, stop=True)
                A = sb0.tile([128, 1], F32, name=f"A{tagp}{jb}",
                             tag=f"A{tagp}{jb}")
                dve.tensor_tensor(A[:, :], pse[:, 0:1], gam, op=OP.mult)
                Bt = sb0.tile([128, 1], F32, name=f"B{tagp}{jb}",
                              tag=f"B{tagp}{jb}")
                muA = sb0.tile([128, 1], F32, name=f"muA{tagp}{jb}",
                               tag=f"muA{tagp}", bufs=2)
                dve.tensor_tensor(muA[:, :], pse[:, 1:2], A[:, :], op=OP.mult)
                dve.tensor_tensor(_r(Bt[:, :]), bet, muA[:, :], op=OP.subtract)
                return A, Bt

            # ---- context groupnorm (materialized, fp8 out) ----
            er_c = chan_stats(lambda jb: ctx_sb[:, jb // 2, jb % 2, :], NCB, 1,
                              S, ind_c, 1.0 / (CC // G), "c")
            for jb in range(NCB):
                A, Bt = expand_ab(er_c, indT_c(jb), jb, vcol("gcg", jb),
                                  vcol("gcb", jb), "c")
                dve.tensor_scalar(gnc8[:, jb // 2, jb % 2, :],
                                  ctx_sb[:, jb // 2, jb % 2, :], A[:, :],
                                  Bt[:, :], op0=OP.mult, op1=OP.add)

            # ---- x8 quantize: blocks 0-1 are produced whole by the ACT
            # stats pass below; gpsimd casts blocks 2-3 tile-by-tile ----
            x8v = x8.rearrange("p a b l -> p (a b) l")
            xallv = xall.rearrange("p a b l -> p (a b) l")

            def x8_cast(ti):
                tsl = ds(ti * TT, TT)
                gp.tensor_copy(x8v[:, 2:4, tsl], xallv[:, 2:4, tsl])

            # ACT: fused cast+sum and square+sum for x blocks 0 and 1
            accx = sb0.tile([128, 2, 2], F32, name="accx", tag="accx")
            scrx = sb0.tile([128, L], BF16, name="scrx", tag="scrx")
            for jb in range(2):
                act.activation(x8v[:, jb, :], xallv[:, jb, :], AF.Copy,
                               accum_out=accx[:, jb, 0:1])
                act.activation(scrx[:, :], xallv[:, jb, :], AF.Square,
                               accum_out=accx[:, jb, 1:2])

            for ti in range(2):
                x8_cast(ti)

            # ---- x stats -> fold into wq8 / bqe ----
            # blocks 0,1 come from the ACT accumulators; 2,3 from bn_stats
            def xr2_act(jb):
                r2 = sb0.tile([128, 2], F32, name=f"r2xa{jb}", tag=f"r2xa{jb}")
                dve.tensor_scalar(r2[:, :], accx[:, jb, :], 1.0 / L, None,
                                  op0=OP.mult)
                return r2
            er_x = chan_stats(lambda jb: xall[:, 1, jb, :], 2, 8,
                              512, ind_x[2:], 1.0 / (C // G), "x",
                              pre_rhs=[(ind_x[0], lambda: xr2_act(0)),
                                       (ind_x[1], lambda: xr2_act(1))])
            Bx = []
            for jb in range(NXB):
                A, Bt = expand_ab(er_x, indT_x(jb), jb, vcol("gxg", jb),
                                  vcol("gxb", jb), "x")
                act.activation(wq8[:, jb // 2, jb % 2, :],
                               wqT_sb[:, jb // 2, jb % 2, :], AF.Copy,
                               scale=A[:, :])
                Bx.append(Bt)
            for b in range(NQB):
                psb = ps0.tile([128, 1], F32, name=f"psb{b}", tag="misc",
                               bufs=2)
                for jb in range(NXB):
                    pe.matmul(psb[:, :],
                              wqT_sb[:, jb // 2, jb % 2,
                                     b * 128:(b + 1) * 128],
                              Bx[jb][:, :], start=(jb == 0),
                              stop=(jb == NXB - 1))
                dve.tensor_tensor(bqe[:, b:b + 1], psb[:, :], vcol("bq", b),
                                  op=OP.add)

            # ---- K and V^T projections (fp8 DoubleRow) ----
            psk = [ps0.tile([128, S], F32, name=f"psk{b}", tag=f"psk{b}")
                   for b in range(NQB)]
            psv = [ps0.tile([128, INNER], F32, name=f"psv{sc}", tag=f"psv{sc}")
                   for sc in range(2)]
            for kc2 in range(3):
                for b in range(NQB):
                    pe.matmul(psk[b][:, :],
                              wkv8[:, kc2, :, b * 128:(b + 1) * 128],
                              gnc8[:, kc2, :, :], start=(kc2 == 0),
                              stop=(kc2 == 2), perf_mode=DR)
                for sc in range(2):
                    pe.matmul(psv[sc][:, :],
                              gnc8[:, kc2, :, sc * 128:(sc + 1) * 128],
                              wkv8[:, kc2, :, INNER:2 * INNER],
                              start=(kc2 == 0), stop=False, perf_mode=DR)
            for sc in range(2):
                pe.matmul(psv[sc][:, :], _r(ones_row), _r(bkvv_row),
                          start=False, stop=True)
            for b in range(NQB):
                act.activation(k8[:, b, :], psk[b][:, :],
                               AF.Identity, bias=vcol("bkvk", b))
            for sc in range(2):
                for par in range(2):
                    dve.tensor_copy(v8[:, sc, par::2, par * DH:(par + 1) * DH],
                                    psv[sc].rearrange("p (h c) -> p h c",
                                                      c=DH)[:, par::2, :])

        # ---------------- t-loop: Q / attention / out-proj ----------------
        with tc.tile_pool(name="work", bufs=1) as work, \
             tc.tile_pool(name="ps1", bufs=1, space="PSUM") as ps1:
            out_view = d["out"].rearrange("(a p b) l -> p a b l", p=128, b=2)
            # Flat cross-tile software pipeline. PSUM (8 banks): psd ring
            # 2x[128,2,TT] (4) + big ring 3x[128,TT] (3, shared by av, bc,
            # psq and pso) + psg32 (1). Heads processed as two quads; the
            # denominator gather lands at psg32 offsets 0/64 (the only legal
            # matmul output base partitions besides 32).
            q8_t = {}
            E_t = {}
            psg_t = {}
            av_t = {}
            avn_t = {}

            def emit_q(t, b):
                tsl = ds(t * TT, TT)
                if b == 0:
                    q8_t[t] = work.tile([128, 4, TT], F8, name=f"q8_{t}",
                                        tag="q8", bufs=3)
                psq = ps1.tile([128, TT], F32, name=f"psq{t}_{b}", tag="qo",
                               bufs=1)
                for kc2 in range(2):
                    pe.matmul(psq[:, :], wq8[:, kc2, :, b * 128:(b + 1) * 128],
                              x8[:, kc2, :, tsl], start=(kc2 == 0),
                              stop=(kc2 == 1), perf_mode=DR)
                dve.tensor_scalar_add(q8_t[t][:, b, :], psq[:, :],
                                      bqe[:, b:b + 1])

            def dots_exp(t, h):
                psd = ps1.tile([128, 2, TT], F32, name=f"psd{t}_{h}",
                               tag="psd", bufs=2)
                h4 = (h % 2) * DH
                q8 = q8_t[t]
                for sc in range(2):
                    pe.matmul(psd[:, sc, :],
                              k8[h4:h4 + DH, h // 2,
                                 sc * 128:(sc + 1) * 128],
                              q8[h4:h4 + DH, h // 2, :],
                              start=True, stop=True)
                E8 = work.tile([128, 2, TT], F8, name=f"E{t}_{h}", tag="E",
                               bufs=4)
                act.activation(E8[:, :, :], psd[:, :, :], AF.Exp,
                               scale=SCALE2)
                E_t[(t, h)] = E8

            def av_mm(t, p2):
                """AV + denominator-gather matmuls for head pair p2. Each
                quad's denominators accumulate into one [36, TT] psum at
                rows {0, 1, 32, 33}; the quad's two av pairs land in one
                [128, 2, TT] psum tile."""
                if p2 == 0:
                    avn_t[t] = work.tile([128, 4, TT], F8, name=f"avn{t}",
                                         tag="avn", bufs=3)
                if p2 % 2 == 0:
                    psg_t[t] = ps1.tile([36, TT], F32,
                                        name=f"psg{t}_{p2 // 2}",
                                        tag="m1", bufs=1)
                psgQ = psg_t[t]
                av = ps1.tile([128, TT], F32, name=f"av{t}_{p2}", tag="av",
                              bufs=2)
                for r2 in range(2):
                    h = 2 * p2 + r2
                    E8 = E_t.pop((t, h))
                    pe.matmul(av[:, :], v8[:, :, h, :], E8[:, :, :],
                              start=(r2 == 0), stop=(r2 == 1), perf_mode=DR)
                    pe.matmul(psgQ[:, :], ones8[:, :, h, :], E8[:, :, :],
                              start=(h % 4 == 0), stop=(h % 4 == 3),
                              perf_mode=DR)
                return av

            def rcp_quad(t, qd):
                rcpQ = work.tile([36, TT], F32, name=f"rcq{t}_{qd}", tag="rcp",
                                 bufs=2)
                with nc.allow_low_precision(reason="f32r bitcast is 32-bit"):
                    dve.reciprocal(_r(rcpQ[:, :]), psg_t[t][:, :])
                return rcpQ

            def norm_pair(t, p2, av, rcpQ):
                """broadcast 1/denom for pair p2 (PE) and normalize its av."""
                bc = ps1.tile([128, TT], F32, name=f"bc{t}_{p2}", tag="m1",
                              bufs=1)
                pe.matmul(bc[:, :], _r(patP[:, p2 % 2, :]), _r(rcpQ[:, :]),
                          start=True, stop=True)
                bcs = work.tile([128, TT], F32, name=f"bcs{t}_{p2}", tag="bcs",
                                bufs=4)
                act.copy(bcs[:, :], bc[:, :])
                dve.tensor_tensor(avn_t[t][:, p2, :], av[:, :], bcs[:, :],
                                  op=OP.mult)

            out_t = {}

            def emit_o(t, m):
                tsl = ds(t * TT, TT)
                if m == 0:
                    out_t[t] = work.tile([128, 2, 2, TT], F32, name=f"o{t}",
                                         tag="osb", bufs=3)
                pso = ps1.tile([128, TT], F32, name=f"pso{t}_{m}", tag="qo",
                               bufs=1)
                for kq2 in range(2):
                    pe.matmul(pso[:, :], wo8[:, kq2, :, m * 128:(m + 1) * 128],
                              avn_t[t][:, 2 * kq2:2 * kq2 + 2, :],
                              start=(kq2 == 0), stop=(kq2 == 1), perf_mode=DR)
                eng = dve
                eng.scalar_tensor_tensor(out_t[t][:, m // 2, m % 2, :],
                                         pso[:, :], vcol("bo", m),
                                         xall[:, m // 2, m % 2, tsl],
                                         op0=OP.add, op1=OP.add)
                if m == 3:
                    ot = out_t.pop(t)
                    for kc2 in range(2):
                        sync.dma_start(out_view[:, kc2, :, tsl],
                                       ot[:, kc2, :, :])
                    avn_t.pop(t)

            # prologue
            for b in range(NQB):
                emit_q(0, b)
            dots_exp(0, 0)
            dots_exp(0, 1)
            for t in range(NT):
                for qd in range(2):
                    pa, pb = 2 * qd, 2 * qd + 1
                    dots_exp(t, 4 * qd + 2)
                    dots_exp(t, 4 * qd + 3)
                    av_a = av_mm(t, pa)
                    if qd == 0:
                        dots_exp(t, 4)
                        dots_exp(t, 5)
                    elif t + 1 < NT:
                        emit_q(t + 1, 0)
                        emit_q(t + 1, 1)
                        dots_exp(t + 1, 0)
                        dots_exp(t + 1, 1)
                    av_b = av_mm(t, pb)
                    rcpQ = rcp_quad(t, qd)
                    norm_pair(t, pa, av_a, rcpQ)
                    norm_pair(t, pb, av_b, rcpQ)
                    if t > 0:
                        emit_o(t - 1, 2 * qd)
                        emit_o(t - 1, 2 * qd + 1)
                if t + 1 < NT:
                    if t + 2 < NT:
                        x8_cast(t + 2)
                    emit_q(t + 1, 2)
                    emit_q(t + 1, 3)
            for m in range(4):
                emit_o(NT - 1, m)


_CACHE = {}


def _build():
    if "nc" in _CACHE:
        return _CACHE["nc"]
    nc = bacc.Bacc("TRN2", target_bir_lowering=False, debug=False,
                   num_devices=NCORES)
    d = {}
    d["x"] = nc.dram_tensor("x", [128, 4 * L], BF16, kind="ExternalInput").ap()
    d["ctx"] = nc.dram_tensor("ctx", [128, 6 * S], BF16,
                              kind="ExternalInput").ap()
    d["wqT"] = nc.dram_tensor("wqT", [128, 4 * INNER], F32,
                              kind="ExternalInput").ap()
    d["wkv8"] = nc.dram_tensor("wkv8", [128, 6 * INNER * 2], F8,
                               kind="ExternalInput").ap()
    d["wo8"] = nc.dram_tensor("wo8", [128, 4 * C], F8,
                              kind="ExternalInput").ap()
    d["vecs"] = nc.dram_tensor("vecs", [128, VCOLS], F32,
                               kind="ExternalInput").ap()
    d["indall"] = nc.dram_tensor("indall", [(NCB + NXB) * 128, G], F32,
                                 kind="ExternalInput").ap()
    d["indTall"] = nc.dram_tensor("indTall", [G, (NCB + NXB) * 128], F32,
                                  kind="ExternalInput").ap()
    d["ones8"] = nc.dram_tensor("ones8", [128, 2 * NH * 36], F8,
                                kind="ExternalInput").ap()
    d["patP"] = nc.dram_tensor("patP", [36, 2 * 128], F32,
                               kind="ExternalInput").ap()
    d["rowm"] = nc.dram_tensor("rowm", [1, 128 + INNER], F32,
                               kind="ExternalInput").ap()
    d["out"] = nc.dram_tensor("out", [C, L], F32, kind="ExternalOutput").ap()

    with tile.TileContext(nc) as tc:
        _emit(nc, tc, d)
    nc.compile()
    _CACHE["nc"] = nc
    return nc


# ---- host-side orderings ----
def _x_chan(kc2, p, j):
    return kc2 * 256 + 2 * p + j


def _q_chan(b, c):
    mq2, jq = b // 2, b % 2
    return (mq2 * 4 + c // 32) * 64 + (c % 32) * 2 + jq


def _host_inputs(inputs):
    f = np.float32
    bf = ml_dtypes.bfloat16
    f8 = ml_dtypes.float8_e4m3fn

    x = np.asarray(inputs["x"], dtype=f).reshape(B, C, L)
    ctx = np.asarray(inputs["context"], dtype=f)
    wq = np.asarray(inputs["wq"], dtype=f)
    wkv = np.asarray(inputs["wkv"], dtype=f)
    wo = np.asarray(inputs["wo"], dtype=f)
    bkv = np.asarray(inputs["bkv"], dtype=f)

    p_ = np.arange(128)
    # x/out channel order: channel(p; kc2, j) = kc2*256 + 2p + j
    xch = np.empty((2, 128, 2), np.int64)
    for kc2 in range(2):
        for j in range(2):
            xch[kc2, :, j] = _x_chan(kc2, p_, j)
    xperm = xch.transpose(1, 0, 2).reshape(128, 4)     # [p, (kc2,j)]
    # ctx channel order
    cch = np.empty((3, 128, 2), np.int64)
    for kc2 in range(3):
        for j in range(2):
            cch[kc2, :, j] = kc2 * 256 + 2 * p_ + j
    cperm = cch.transpose(1, 0, 2).reshape(128, 6)     # [p, (kc2,j)]
    # q/k inner order: natural (head h at rows (h%2)*64 of block h//2)
    qcols = np.arange(NQB * 128).reshape(NQB, 128)
    qorder = qcols.reshape(-1)

    # x_dev [128, (kc2,j,L)] bf16
    x_dev = np.empty((B, 128, 2, 2, L), bf)
    for kc2 in range(2):
        for j in range(2):
            x_dev[:, :, kc2, j, :] = x[:, xch[kc2, :, j], :].astype(bf)
    x_dev = x_dev.reshape(B, 128, 4 * L)

    # ctx_dev [128, (kc2,j,S)] bf16
    ctx_dev = np.empty((B, 128, 3, 2, S), bf)
    for kc2 in range(3):
        for j in range(2):
            ctx_dev[:, :, kc2, j, :] = ctx[:, cch[kc2, :, j], :].astype(bf)
    ctx_dev = ctx_dev.reshape(B, 128, 6 * S)

    # wqT_dev [128, (kc2,j,512cols)] f32 : wq[qorder(col), xchan(p,kc2,j)]
    wqT_dev = np.empty((128, 2, 2, INNER), f)
    for kc2 in range(2):
        for j in range(2):
            wqT_dev[:, kc2, j, :] = wq[np.ix_(qorder, xch[kc2, :, j])].T
    wqT_dev = wqT_dev.reshape(128, 4 * INNER)

    # wkv8_dev [128, (kc2,j, k512 | v512)] fp8
    wkv8_dev = np.empty((128, 3, 2, 2 * INNER), f)
    vorder = np.arange(INNER) + INNER        # natural v rows of wkv
    for kc2 in range(3):
        for j in range(2):
            cc = cch[kc2, :, j]
            wkv8_dev[:, kc2, j, :INNER] = wkv[np.ix_(qorder, cc)].T
            wkv8_dev[:, kc2, j, INNER:] = wkv[np.ix_(vorder, cc)].T
    wkv8_dev = wkv8_dev.reshape(128, 12 * INNER).astype(f8)

    # wo8_dev [128, (kq2,jq, 512cols)] fp8 : wo[outchan(col), (2kq2+jq)*128+p]
    outcols = np.empty((4, 128), np.int64)
    for bo in range(4):
        outcols[bo] = _x_chan(bo // 2, np.arange(128), bo % 2)
    wo8_dev = np.empty((128, 2, 2, C), f)
    for kq2 in range(2):
        for jq in range(2):
            inner_idx = (2 * kq2 + jq) * 128 + p_
            wo8_dev[:, kq2, jq, :] = wo[np.ix_(outcols.reshape(-1),
                                               inner_idx)].T
    wo8_dev = wo8_dev.reshape(128, 4 * C).astype(f8)

    # indicator matrices (permuted orders)
    ind_x = np.zeros((NXB, 128, G), f)
    indT_x = np.zeros((NXB, G, 128), f)
    for blk in range(NXB):
        g = xch[blk // 2, :, blk % 2] // (C // G)
        ind_x[blk, p_, g] = 1.0
        indT_x[blk, g, p_] = 1.0
    ind_c = np.zeros((NCB, 128, G), f)
    indT_c = np.zeros((NCB, G, 128), f)
    for blk in range(NCB):
        g = cch[blk // 2, :, blk % 2] // (CC // G)
        ind_c[blk, p_, g] = 1.0
        indT_c[blk, g, p_] = 1.0
    indall = np.concatenate([ind_c, ind_x], axis=0).reshape(-1, G)
    indall = np.ascontiguousarray(indall)
    indTall = np.ascontiguousarray(
        np.concatenate([indT_c, indT_x], axis=0).transpose(1, 0, 2)
        .reshape(G, -1))

    # gather patterns: quad pairs land at rows {0,1} and {32,33}.
    # Unused rows also get a (real) denominator so 1/x never sees a zero.
    ones8 = np.zeros((128, 2, NH, 36), f)
    for h in range(NH):
        ones8[:, :, h, 32 * ((h % 4) // 2) + h % 2] = 1.0
        if h % 4 == 0:
            ones8[:, :, h, 2:32] = 1.0
        if h % 4 == 2:
            ones8[:, :, h, 34:36] = 1.0
    ones8 = ones8.reshape(128, 2 * NH * 36).astype(f8)

    # patP[r, pq, c] = (r == 32*pq + (c >= 64))
    patP = np.zeros((36, 2, 128), f)
    for pq in range(2):
        patP[32 * pq, pq, 0:DH] = 1.0
        patP[32 * pq + 1, pq, DH:128] = 1.0
    patP = patP.reshape(36, 2 * 128)


    def cols_perm(vec, order):
        return np.asarray(vec, dtype=f)[order]          # [128, n]

    vecs = np.zeros((128, VCOLS), f)
    vecs[:, VOFF["bq"]:VOFF["bq"] + 4] = cols_perm(inputs["bq"], qcols.T)
    vecs[:, VOFF["bkvk"]:VOFF["bkvk"] + 4] = cols_perm(bkv[:INNER], qcols.T)
    vecs[:, VOFF["bo"]:VOFF["bo"] + 4] = cols_perm(inputs["bo"],
                                                   outcols.T)
    vecs[:, VOFF["gxg"]:VOFF["gxg"] + 4] = cols_perm(inputs["gnx_g"], xperm)
    vecs[:, VOFF["gxb"]:VOFF["gxb"] + 4] = cols_perm(inputs["gnx_b"], xperm)
    vecs[:, VOFF["gcg"]:VOFF["gcg"] + 6] = cols_perm(inputs["gnc_g"], cperm)
    vecs[:, VOFF["gcb"]:VOFF["gcb"] + 6] = cols_perm(inputs["gnc_b"], cperm)
    vecs[:, VOFF["eps"]] = EPS

    rowm = np.zeros((1, 128 + INNER), f)
    rowm[0, :128] = 1.0
    rowm[0, 128:] = bkv[INNER:]

    shared = {
        "wqT": wqT_dev,
        "wkv8": wkv8_dev,
        "wo8": wo8_dev,
        "indall": indall,
        "indTall": indTall,
        "ones8": ones8,
        "patP": patP,
        "rowm": rowm,
        "vecs": vecs,
    }
    return [{"x": x_dev[i], "ctx": ctx_dev[i], **shared} for i in range(NCORES)]


def run(inputs, **spmd_kwargs):
    nc = _build()
    in_maps = _host_inputs(inputs)
    res = run_bass_kernel_spmd(nc, in_maps, list(range(NCORES)), **spmd_kwargs)
    out = np.stack([np.asarray(res.results[i]["out"], dtype=np.float32)
                    for i in range(NCORES)])
    return out.reshape(B, C, 64, 64), res


def kernel(**inputs) -> np.ndarray:
    out, _ = run(inputs)
    return out
